# revision 1
# baseline (speedup 1.0000x reference)
"""Trainium2 Bass kernel for nn_DemandMap (histogram_binning).

Math: the scatter-add histogram is a dense separable 8x8 block reduction:
hist_t = WX_t^T @ mask_t @ WY_t with banded weights by (residue, type).

Stage A runs *transposed* on the PE (mask is the stationary operand), so no
DMA transpose and only narrow PSUM drains are needed:

  input: per j-tile T and word n, an int16 packing the 3 mask bits of TWO
         sites: bit (2+2t) = [type(x=2n)==t+1], bit (10+2t) = [type(x=2n+1)
         ==t+1].  One DVE pass per (tile block, type) -- (>>2t, &0x0404),
         both bitvec ops, all operands 2-byte so the 4x DVE mode applies --
         yields bytes in {0x00, 0x04}; bitcast to fp8e5m2 that is exactly
         2^-14 * mask, with wy pre-scaled by 2^14 (exact).  The last
         POOL_TILES j-tiles instead get is_equal masks (value 1.0, fp16) on
         the gpsimd engine from a raw site-type side plane, with unscaled wy.
  stage A-T: psAT[i, (T,t,q)] = mask_t[:, x-chunk].T @ wy_t -- the moving
         operand is only 15 columns so PE cost is tiny (and LDWEIGHTS is
         free); the output is already x-major.
  drain: psAT f32 -> fp16 AT (exact: values are 0.5-step <= 40), spread
         across ACT/DVE/Pool.
  stage B: psB[65, (t,T,q)] += wx_c^T @ AT_c accumulated over the 4 x-chunks.
  assembly: 64 - psB into the output planes; row 64 is the kx=1 spill into
         the next core's first x-bin, merged on the host.  Output DMA per
         round.

Sharding: core c owns x rows [512c, 512c+512) and bins bx in [64c, 64c+64).
"""

import json
import os

import numpy as np

F16 = np.float16

NCORES = 8
RPAD = 512   # site x cols per core
JPAD = 4224  # 8 top zero pad + 4096 + tail pad (y axis, partition source)
NT = 35      # j-tiles, stride 120, each owns 15 y-bins
NTYPES = 3
PAIR_AND = 0x0404    # surviving bits -> fp8e5m2 2^-14 in each byte
WYSCALE = 2.0 ** 14

POOL_TILES = int(os.environ.get("KERNEL_POOL_TILES", "3"))
ROUNDS = [int(x) for x in os.environ.get("KERNEL_ROUNDS", "10,10,10,5").split(",")]
assert sum(ROUNDS) == NT and all(l <= 11 for l in ROUNDS)
# engine for paired drain (r, pair): A=ACT, D=DVE (Pool cannot access PSUM)
DRAIN_ENG = os.environ.get("KERNEL_DRAIN", "AA,AA,AA,DD").split(",")
# engine for the psB->outp copy per round (last char also covers tail copy)
COPY_ENG = os.environ.get("KERNEL_COPY", "AAAD")
# engine issuing each round's output DMA
ODMA_ENG = os.environ.get("KERNEL_ODMA", "AAAS")
# DVE pass blocks: comma list of DVE-tile counts (must cover ntd and align
# with round boundaries); empty = one block per round
BLOCKS = os.environ.get("KERNEL_BLOCKS", "")
# emit stage B/copy/dma of round r one round late (softens engine queues)
DELAY_TAIL = int(os.environ.get("KERNEL_DELAY", "1"))
# split the first wbits DMA after this many tiles (0 = no split)
R0SPLIT = int(os.environ.get("KERNEL_R0SPLIT", "4"))

_PROG_CACHE = {}
_WSPLIT_DONE = [False]


def _install_wait_split():
    """This walrus build accepts only ONE sync wait per instruction; Tile
    attaches N.  Rewrite the BIR JSON: hoist all-but-one wait onto fresh
    same-engine EventSemaphore waits inserted before the offender."""
    if _WSPLIT_DONE[0]:
        return
    import concourse.bass as bass

    orig = bass.Bass.to_json_bytes

    def split(self, *a, **k):
        m = json.loads(orig(self, *a, **k))
        n = [0]
        for fn in m["functions"]:
            for blk in fn["blocks"]:
                out = []
                for ins in blk["instructions"]:
                    si = ins.get("sync_info")
                    waits = si.get("on_wait") if si else None
                    if waits and len(waits) > 1:
                        for w in waits[:-1]:
                            n[0] += 1
                            nop = {
                                "engine": ins["engine"],
                                "ins": [],
                                "outs": [],
                                "name": f"WSPLIT-{n[0]}",
                                "opcode": "EventSemaphore",
                                "sync_info": {"on_update": [], "on_wait": [w]},
                            }
                            if "debug" in ins:
                                nop["debug"] = ins["debug"]
                            out.append(nop)
                        si["on_wait"] = [waits[-1]]
                    out.append(ins)
                blk["instructions"] = out
        return json.dumps(m).encode()

    bass.Bass.to_json_bytes = split
    _WSPLIT_DONE[0] = True


def _oxy_weights(size_f32):
    """Per-residue overlap weights, matching the reference f32 formulas."""
    r = np.arange(8, dtype=np.float32)
    o0 = np.maximum(np.minimum(np.float32(8.0) - r, size_f32), np.float32(0.0))
    o1 = np.maximum(
        np.minimum(np.float32(16.0) - r, size_f32) - (np.float32(8.0) - r),
        np.float32(0.0),
    )
    return o0.astype(np.float32), o1.astype(np.float32)


def _build_wy(sy):
    """f32 [128, 3, 15]: y-stage stationary (y-halo tile form), types 1..3.
    Tile partition p holds padded j = 120T + p (jp = j + 8), so ry = p % 8;
    bin q gets ky=0 rows p//8 == q+1 and ky=1 rows p//8 == q."""
    W = np.zeros((128, 3, 15), np.float32)
    for tp in range(3):
        o0, o1 = _oxy_weights(np.float32(sy[tp + 1]))
        for p in range(128):
            if 0 <= p // 8 - 1 < 15:
                W[p, tp, p // 8 - 1] += o0[p % 8]
            if p // 8 < 15:
                W[p, tp, p // 8] += o1[p % 8]
    return W


def _build_wx(sx):
    """f32 [512, 3, 65]: x-stage stationary.  Local row r = x - 512c; col 64
    is the spill bin (kx=1 of the last 8 rows -> next core's first bin)."""
    W = np.zeros((RPAD, 3, 65), np.float32)
    for tp in range(3):
        o0, o1 = _oxy_weights(np.float32(sx[tp + 1]))
        for r in range(512):
            W[r, tp, r // 8] += o0[r % 8]
            W[r, tp, r // 8 + 1] += o1[r % 8]
    return W


def _hi_lo16(w):
    hi = w.astype(F16)
    lo = (w - hi.astype(np.float32)).astype(F16)
    return hi, lo


def _build_program(use_ylo, use_xlo, ntd):
    _install_wait_split()
    import concourse.bass as bass
    import concourse.mybir as mybir
    from concourse.tile import TileContext
    from contextlib import ExitStack

    dt = mybir.dt
    nc = bass.Bass()
    npool = NT - ntd
    Lmax = max(ROUNDS)
    NR = len(ROUNDS)

    WBITS = nc.declare_dram_parameter("wbits", [128, ntd * 128], dt.int16, isOutput=False)
    if npool:
        STP = nc.declare_dram_parameter("stp", [128, npool * 512], dt.int16, isOutput=False)
    # weight pack (fp16): wy hi scaled [0:48], wy lo scaled [48:96],
    # wy hi unscaled [96:144], wy lo unscaled [144:192],
    # wx hi 192+65*(3c+t) (negated); wx lo after (if xlo);
    # then w64 [65] = 0.5 and wones [15*Lmax] = 1.0 for the +64 bias matmul
    WXO = 192
    WXE = WXO + 780 + (780 if use_xlo else 0)
    WTOT = WXE + 65 + 15 * Lmax
    WPACK = nc.declare_dram_parameter("wpack", [128, WTOT], dt.float16, isOutput=False)
    OUT = nc.declare_dram_parameter("outbuf", [65, 3 * 512], dt.float32, isOutput=True)

    with ExitStack() as ctx:
        tc = ctx.enter_context(TileContext(nc))
        pp = ctx.enter_context(tc.tile_pool(name="persist", bufs=1))
        pA = ctx.enter_context(tc.tile_pool(name="psA", bufs=3, space="PSUM"))
        pB = ctx.enter_context(tc.tile_pool(name="psB", bufs=2, space="PSUM"))

        wbig = pp.tile([128, ntd * 128], dt.int16, name="wbig", tag="wbig")
        stp = (
            pp.tile([128, npool * 512], dt.int16, name="stp", tag="stp")
            if npool
            else None
        )
        wpk = pp.tile([128, WTOT], dt.float16, name="wpk", tag="wpk")
        m = [
            pp.tile([128, ntd * 256], dt.int16, name=f"m{t}", tag=f"m{t}")
            for t in range(NTYPES)
        ]
        mp = [
            pp.tile([128, npool * 512], dt.float16, name=f"mp{t}", tag=f"mp{t}")
            for t in range(NTYPES)
        ] if npool else None
        # AT layout: (r, c, Tl, t, q): round r at cols 180*T0(r), chunk c at
        # +45*L*c
        AT = pp.tile([128, NT * 180], dt.float16, name="atb", tag="atb")
        outp = pp.tile([65, 3 * 512], dt.float32, name="outp", tag="outp")

        # ---- DMAs -------------------------------------------------------
        bounds = np.cumsum([0] + ROUNDS)
        dve_rng = []  # DVE tile range per round
        for r in range(NR):
            dve_rng.append((bounds[r], min(bounds[r + 1], ntd)))

        def bits_dma_rng(a, b):
            if b > a:
                nc.sync.dma_start(
                    out=wbig[:, 128 * a: 128 * b], in_=WBITS[:, 128 * a: 128 * b]
                )

        def bits_dma(r):
            bits_dma_rng(*dve_rng[r])

        a0, b0 = dve_rng[0]
        if R0SPLIT and a0 + R0SPLIT < b0:
            bits_dma_rng(a0, a0 + R0SPLIT)
            bits_dma_rng(a0 + R0SPLIT, b0)
        else:
            bits_dma(0)
        if npool:
            nc.sync.dma_start(out=stp[:, :], in_=STP[:, :])
        nc.sync.dma_start(out=wpk[:, :], in_=WPACK[:, :])
        for r in range(1, NR):
            bits_dma(r)

        # ---- masks ------------------------------------------------------
        # Pool is_equal masks (value 1.0 fp16) for tiles [ntd, NT)
        for Ti in range(npool):
            for t in range(NTYPES):
                nc.gpsimd.tensor_scalar(
                    mp[t][:, 512 * Ti: 512 * Ti + 512],
                    stp[:, 512 * Ti: 512 * Ti + 512],
                    float(t + 1),
                    None,
                    mybir.AluOpType.is_equal,
                )

        def dve_block(a, b):
            if b <= a:
                return
            win = wbig[:, 128 * a: 128 * b]
            for v in range(2):     # v-major: x-chunk pair 0 unlocks early
                for t in range(NTYPES):
                    k = 2 * t + v
                    dst = m[t].rearrange("p (T vv n) -> p T vv n", vv=2, n=128)[
                        :, a:b, v, :
                    ]
                    if k == 0:
                        nc.vector.tensor_scalar(
                            dst, win, PAIR_AND, None, mybir.AluOpType.bitwise_and
                        )
                    else:
                        nc.vector.tensor_scalar(
                            dst, win, k, PAIR_AND,
                            mybir.AluOpType.logical_shift_right,
                            op1=mybir.AluOpType.bitwise_and,
                        )

        # weight views
        wy_hi_s = [wpk[:, 16 * t: 16 * t + 15] for t in range(NTYPES)]
        wy_lo_s = [wpk[:, 48 + 16 * t: 48 + 16 * t + 15] for t in range(NTYPES)]
        wy_hi_u = [wpk[:, 96 + 16 * t: 96 + 16 * t + 15] for t in range(NTYPES)]
        wy_lo_u = [wpk[:, 144 + 16 * t: 144 + 16 * t + 15] for t in range(NTYPES)]
        wx_hi = [
            [wpk[:, WXO + 65 * (3 * c + t): WXO + 65 * (3 * c + t) + 65] for t in range(NTYPES)]
            for c in range(4)
        ]
        wx_lo = (
            [
                [
                    wpk[:, WXO + 780 + 65 * (3 * c + t): WXO + 780 + 65 * (3 * c + t) + 65]
                    for t in range(NTYPES)
                ]
                for c in range(4)
            ]
            if use_xlo
            else None
        )
        w64 = wpk[:, WXE: WXE + 65]
        wones = wpk[:, WXE + 65: WXE + 65 + 15 * Lmax]

        def drain(eng, dst, src):
            if eng == "A":
                nc.scalar.copy(dst, src)
            else:
                nc.vector.tensor_copy(out=dst, in_=src)

        def emit_matmuls_and_drains(r):
            L = ROUNDS[r]
            T0 = bounds[r]
            atr = 180 * T0
            for pair in range(2):
                psA = pA.tile([128, 1024], dt.float32, name="psA")
                for cc in range(2):
                    c = 2 * pair + cc
                    for Tl in range(L):
                        T = T0 + Tl
                        for t in range(NTYPES):
                            if T < ntd:
                                lhs = m[t][:, 256 * T + 64 * c: 256 * T + 64 * c + 64].bitcast(
                                    dt.float8e5
                                )
                                rh, rl = wy_hi_s[t], wy_lo_s[t]
                            else:
                                Ti = T - ntd
                                lhs = mp[t][:, 512 * Ti + 128 * c: 512 * Ti + 128 * c + 128]
                                rh, rl = wy_hi_u[t], wy_lo_u[t]
                            dst = psA[:, 512 * cc + 45 * Tl + 15 * t: 512 * cc + 45 * Tl + 15 * t + 15]
                            if use_ylo:
                                nc.tensor.matmul(dst, lhsT=lhs, rhs=rh, start=True, stop=False)
                                nc.tensor.matmul(dst, lhsT=lhs, rhs=rl, start=False, stop=True)
                            else:
                                nc.tensor.matmul(dst, lhsT=lhs, rhs=rh, start=True, stop=True)
                drain(
                    DRAIN_ENG[r][pair],
                    AT[:, atr + 90 * L * pair: atr + 90 * L * pair + 90 * L],
                    psA.rearrange("p (two x) -> p two x", two=2)[:, :, 0: 45 * L],
                )

        def emit_tail(r):
            L = ROUNDS[r]
            T0 = bounds[r]
            atr = 180 * T0
            # stage B: psB cols (t, Tl, q)
            psB = pB.tile([65, 45 * Lmax], dt.float32, name="psB")
            for t in range(NTYPES):
                dstB = psB[:, 15 * Lmax * t: 15 * Lmax * t + 15 * L].rearrange(
                    "p (T q) -> p T q", q=15
                )
                for c in range(4):
                    rhs = AT[:, atr + 45 * L * c: atr + 45 * L * c + 45 * L].rearrange(
                        "p (T g q) -> p T g q", g=3, q=15
                    )[:, :, t, :]
                    nc.tensor.matmul(
                        dstB, lhsT=wx_hi[c][t], rhs=rhs,
                        start=(c == 0), stop=False,
                    )
                if use_xlo:
                    for c in range(4):
                        rhs = AT[:, atr + 45 * L * c: atr + 45 * L * c + 45 * L].rearrange(
                            "p (T g q) -> p T g q", g=3, q=15
                        )[:, :, t, :]
                        nc.tensor.matmul(
                            dstB, lhsT=wx_lo[c][t], rhs=rhs,
                            start=False, stop=False,
                        )
                # +64 bias (wx is negated): psB = 64 - hist
                nc.tensor.matmul(
                    dstB, lhsT=w64, rhs=wones[:, 0: 15 * L].rearrange(
                        "p (T q) -> p T q", q=15
                    ),
                    start=False, stop=True,
                )
            # stage psB (already 64 - hist) to SBUF, then per-round DMA out
            nfull = L if T0 + L < NT else L - 1
            ceng = COPY_ENG[r]
            ov = outp.rearrange("p (g y) -> p g y", y=512)
            pv = psB.rearrange("p (g x) -> p g x", x=15 * Lmax)
            if ceng == "A":
                nc.scalar.copy(ov[:, :, 15 * T0: 15 * (T0 + nfull)], pv[:, :, 0: 15 * nfull])
            else:
                nc.vector.tensor_copy(
                    out=ov[:, :, 15 * T0: 15 * (T0 + nfull)], in_=pv[:, :, 0: 15 * nfull]
                )
            if nfull < L:  # tail tile T=34: only bins 510, 511
                if ceng == "A":
                    nc.scalar.copy(ov[:, :, 510:512], pv[:, :, 15 * (L - 1): 15 * (L - 1) + 2])
                else:
                    nc.vector.tensor_copy(
                        out=ov[:, :, 510:512], in_=pv[:, :, 15 * (L - 1): 15 * (L - 1) + 2]
                    )
            lastc = 512 if T0 + L == NT else 15 * (T0 + nfull)
            deng = nc.scalar if ODMA_ENG[r] == "A" else (
                nc.vector if ODMA_ENG[r] == "D" else nc.sync
            )
            deng.dma_start(
                out=OUT.rearrange("p (g y) -> p g y", y=512)[:, :, 15 * T0: lastc],
                in_=ov[:, :, 15 * T0: lastc],
            )

        # DVE pass blocks (eager, gated only by input DMAs)
        if BLOCKS:
            blks = [int(x) for x in BLOCKS.split(",")]
            assert sum(blks) == ntd
            a = 0
            for bl in blks:
                dve_block(a, a + bl)
                a += bl
        else:
            for r in range(NR):
                dve_block(*dve_rng[r])

        for r in range(NR):
            emit_matmuls_and_drains(r)
            if DELAY_TAIL and r > 0:
                emit_tail(r - 1)
            elif not DELAY_TAIL:
                emit_tail(r)
        if DELAY_TAIL:
            emit_tail(NR - 1)
    return nc

def _get_program(use_ylo, use_xlo, ntd):
    key = (use_ylo, use_xlo, ntd)
    if key not in _PROG_CACHE:
        _PROG_CACHE[key] = _build_program(use_ylo, use_xlo, ntd)
    return _PROG_CACHE[key]


def kernel(site_type_map, site_size_x, site_size_y):
    from concourse.bass_utils import run_bass_kernel_spmd

    smap = np.asarray(site_type_map, dtype=np.int32)
    sx = np.asarray(site_size_x, dtype=np.float32)
    sy = np.asarray(site_size_y, dtype=np.float32)

    WYf = _build_wy(sy)  # [128, 3, 15]
    WXf = _build_wx(sx)  # [512, 3, 65]
    wy_hi_s, wy_lo_s = _hi_lo16(WYf * np.float32(WYSCALE))
    wy_hi_u, wy_lo_u = _hi_lo16(WYf)
    wx_hi, wx_lo = _hi_lo16(-WXf)  # negated: psB accumulates 64 - hist
    use_ylo = bool(
        np.any(wy_lo_s.astype(np.float32) != 0) or np.any(wy_lo_u.astype(np.float32) != 0)
    )
    use_xlo = bool(np.any(wx_lo.astype(np.float32) != 0))
    ntd = NT - POOL_TILES
    Lmax = max(ROUNDS)

    WXO = 192
    WXE = WXO + 780 + (780 if use_xlo else 0)
    WTOT = WXE + 65 + 15 * Lmax
    wpk = np.zeros((128, WTOT), F16)
    for t in range(3):
        wpk[:, 16 * t: 16 * t + 15] = wy_hi_s[:, t, :]
        wpk[:, 48 + 16 * t: 48 + 16 * t + 15] = wy_lo_s[:, t, :]
        wpk[:, 96 + 16 * t: 96 + 16 * t + 15] = wy_hi_u[:, t, :]
        wpk[:, 144 + 16 * t: 144 + 16 * t + 15] = wy_lo_u[:, t, :]
        for c in range(4):
            o = WXO + 65 * (3 * c + t)
            wpk[:, o: o + 65] = wx_hi[128 * c: 128 * c + 128, t, :]
            if use_xlo:
                o2 = WXO + 780 + 65 * (3 * c + t)
                wpk[:, o2: o2 + 65] = wx_lo[128 * c: 128 * c + 128, t, :]
    wpk[:, WXE: WXE + 65] = np.float16(0.5)       # w64: 128 * 0.5 = 64
    wpk[:, WXE + 65: WXE + 65 + 15 * Lmax] = np.float16(1.0)  # wones

    nc = _get_program(use_ylo, use_xlo, ntd)

    in_maps = []
    for c in range(NCORES):
        sjp = np.zeros((JPAD, RPAD), np.int16)
        sjp[8: 8 + 4096, :] = smap[512 * c: 512 * c + 512].T
        wbits = np.empty((128, ntd * 128), np.int16)
        stp_l = []
        for T in range(NT):
            blk = sjp[120 * T: 120 * T + 128, :]
            if T < ntd:
                w = np.zeros((128, 128), np.uint16)
                for tt in (1, 2, 3):
                    # x = 256v + 2n + h -> bit (2*tt + v + 8h) of word n
                    mk = (blk == tt).astype(np.uint16).reshape(128, 2, 128, 2)
                    for v in range(2):
                        for h in range(2):
                            w |= mk[:, v, :, h] << (2 * tt + v + 8 * h)
                wbits[:, 128 * T: 128 * T + 128] = w.view(np.int16)
            else:
                stp_l.append(blk)
        mm = {"wbits": wbits, "wpack": wpk}
        if stp_l:
            mm["stp"] = np.concatenate(stp_l, axis=1)
        in_maps.append(mm)

    res = run_bass_kernel_spmd(
        nc,
        in_maps,
        core_ids=list(range(NCORES)),
        trace=bool(int(os.environ.get("KERNEL_TRACE", "0"))),
    )
    kernel._last_results = res

    # device returns 3 type planes; comp2site=(1,1,2,3) duplicates plane 0
    full = np.empty((4, 512, 512), np.float32)
    for c in range(NCORES):
        ob = res.results[c]["outbuf"]  # [65, 1536]
        for t in range(3):
            full[t + 1, 64 * c: 64 * c + 64, :] = ob[0:64, 512 * t: 512 * t + 512]
    for c in range(NCORES - 1):
        ob = res.results[c]["outbuf"]
        for t in range(3):
            # spill row: (64-h0) + (64-h1) - 64 = 64 - h0 - h1
            full[t + 1, 64 * (c + 1), :] += ob[64, 512 * t: 512 * t + 512] - np.float32(64.0)
    full[0] = full[1]
    return full

